# revision 1
# baseline (speedup 1.0000x reference)
"""Trainium2 Bass kernel for CrossBranchAttentionWithSA.

Sharding: 8 cores = 2 batches x 4 query-chunks of 576 OWN queries (no halo).
The 7x7 SpatialAttention conv needs neighbor rows only through the 2-channel
mean/max stats, so each core computes attention/proj for exactly its own 576
queries and the per-query stats are exchanged with a tiny 4-rank AllGather
([2,640] f32 per core); halo stat rows are then fetched from the gathered
buffer with partition-id-dependent (dynamic-offset, cond-predicated) DMAs.

Device schedule (per core):
 1. K tile 0 + Q tile 0 projections first so attention starts early; V (with
    per-head 65th ones column yielding the softmax denominator) and remaining
    K/Q tiles are emitted between attention heads. Projection PSUM uses
    1-bank [128,512] chunks (bufs=2) so it coexists with attention PSUM
    (st 2x2 banks + av 2 banks = 8 total) instead of serializing phases.
 2. Per head: S.T = K_h.T^T Q_h.T -> exp (scale folded) -> AV.T accumulated
    over 18 key tiles; denominator division via DRAM-roundtrip row broadcast.
 3. proj consumes attn.T as stationary giving [query, channel] rows; mean/max
    stats via free-dim reduces -> [128,10] -> DMA-xbar transpose -> DRAM ->
    AllGather -> gutter-padded conv rows -> 7 shifted K=14 matmuls ->
    sigmoid -> per-query scale -> store.
"""
import os
import numpy as np
import ml_dtypes

import concourse.bass as bass
import concourse.bacc as bacc
import concourse.tile as tile
from concourse import mybir
from concourse.bass_utils import run_bass_kernel_spmd

F32 = mybir.dt.float32
BF16 = mybir.dt.bfloat16
AF = mybir.ActivationFunctionType
AX = mybir.AxisListType
bf16 = ml_dtypes.bfloat16

DIM, HEADS, HGT, WID = 768, 12, 48, 48
HD = DIM // HEADS          # 64
N = HGT * WID              # 2304
SA_K = 7
B = 2
W = 576                    # own queries per core (12 image rows)
ROWS_W = W // WID          # 12
MC = WID + 6               # 54 (gutter-padded row width)
MPW = (ROWS_W + 6) * MC + 6   # 978: 3+12+3 rows plus aprime read tail
CONV_SPAN = ROWS_W * MC    # 648
STATC = 640                # padded stats row (576 valid + 64 zero pad)

SCALE = float(HD) ** -0.5


def build_program():
    nc = bacc.Bacc("TRN2", target_bir_lowering=False, debug=False,
                   enable_asserts=False, num_devices=8)

    xq_t = nc.dram_tensor("xq_t", [DIM, W], BF16, kind="ExternalInput").ap()
    xkv_t = nc.dram_tensor("xkv_t", [DIM, N], BF16, kind="ExternalInput").ap()
    wq_t = nc.dram_tensor("wq_t", [DIM, DIM], BF16, kind="ExternalInput").ap()
    wk_t = nc.dram_tensor("wk_t", [DIM, DIM], BF16, kind="ExternalInput").ap()
    wv_t = nc.dram_tensor("wv_t", [DIM, DIM], BF16, kind="ExternalInput").ap()
    wp_t = nc.dram_tensor("wp_t", [DIM, DIM], BF16, kind="ExternalInput").ap()
    qb_d = nc.dram_tensor("qb", [6, 128], F32, kind="ExternalInput").ap()
    kb_d = nc.dram_tensor("kb", [6, 128], F32, kind="ExternalInput").ap()
    vb_d = nc.dram_tensor("vb", [1, DIM], F32, kind="ExternalInput").ap()
    pb_d = nc.dram_tensor("pb", [1, DIM], F32, kind="ExternalInput").ap()
    saw_d = nc.dram_tensor("saw", [14, SA_K], F32, kind="ExternalInput").ap()
    out_d = nc.dram_tensor("out", [W, DIM], F32, kind="ExternalOutput").ap()

    with tile.TileContext(nc) as tc:
        build_tile(tc, xq_t, xkv_t, wq_t, wk_t, wv_t, wp_t,
                   qb_d, kb_d, vb_d, pb_d, saw_d, out_d)
    nc.compile()
    return nc


def build_tile(tc, xq_t, xkv_t, wq_t, wk_t, wv_t, wp_t,
               qb_d, kb_d, vb_d, pb_d, saw_d, out_d):
    nc = tc.nc

    with tc.tile_pool(name="big", bufs=1) as big:
        # ---------- load inputs (K-path tensors first: K0/Q0 gate head 0) ---
        ins_pool = tc.tile_pool(name="ins", bufs=1)
        ins = ins_pool.__enter__()
        wk_sb = ins.tile([128, 6, DIM], BF16, tag="wk")
        nc.sync.dma_start(wk_sb[:], wk_t.rearrange("(t p) m -> p t m", p=128))
        xkv_ps = [ins.tile([128, 6, 768], BF16, tag="xkv%d" % c,
                           name="xkv%d" % c) for c in range(3)]
        nc.sync.dma_start(
            xkv_ps[0][:],
            xkv_t.rearrange("(t p) m -> p t m", p=128)[:, :, 0:768])
        xq_sb = ins.tile([128, 6, W], BF16, tag="xq")
        nc.sync.dma_start(xq_sb[:], xq_t.rearrange("(t p) m -> p t m", p=128))
        wq_sb = ins.tile([128, 6, DIM], BF16, tag="wq")
        nc.sync.dma_start(wq_sb[:], wq_t.rearrange("(t p) m -> p t m", p=128))
        for c in (1, 2):
            nc.sync.dma_start(
                xkv_ps[c][:],
                xkv_t.rearrange("(t p) m -> p t m",
                                p=128)[:, :, 768 * c:768 * (c + 1)])
        wv_sb = ins.tile([128, 6, DIM], BF16, tag="wv")
        for c in range(2):
            nc.sync.dma_start(
                wv_sb[:, 3 * c:3 * (c + 1)],
                wv_t.rearrange("(t p) m -> p t m", p=128)[:, 3 * c:3 * (c + 1)])
        wp_sb = big.tile([128, 6, DIM], BF16, tag="wp")
        nc.sync.dma_start(wp_sb[:], wp_t.rearrange("(t p) m -> p t m", p=128))

        qb_sb = big.tile([128, 6], F32, tag="qb")
        nc.sync.dma_start(qb_sb[:], qb_d.rearrange("t p -> p t"))
        kb_sb = big.tile([128, 6], F32, tag="kb")
        nc.sync.dma_start(kb_sb[:], kb_d.rearrange("t p -> p t"))
        saw_sb = big.tile([14, SA_K], F32, tag="saw")
        nc.sync.dma_start(saw_sb[:], saw_d)
        vb_bc = big.tile([128, DIM], F32, tag="vbb")
        nc.sync.dma_start(vb_bc[:], bass.AP(
            tensor=vb_d.tensor, offset=0, ap=[[0, 128], [1, DIM]]))
        pb_bc = big.tile([128, DIM], F32, tag="pbb")
        nc.sync.dma_start(pb_bc[:], bass.AP(
            tensor=pb_d.tensor, offset=0, ap=[[0, 128], [1, DIM]]))

        # pre-touch DMA-loaded tiles on DVE+ACT so later instructions inherit
        # the DMA sem ticks instead of each re-waiting
        touch = big.tile([128, 4], F32, tag="touch")
        for ap in (xq_sb[:, 0, 0:2], xkv_ps[0][:, 0, 0:2],
                   xkv_ps[1][:, 0, 0:2], xkv_ps[2][:, 0, 0:2],
                   wq_sb[:, 0, 0:2], wk_sb[:, 0, 0:2], wv_sb[:, 0, 0:2],
                   wp_sb[:, 0, 0:2], qb_sb[:, 0:2], kb_sb[:, 0:2],
                   vb_bc[:, 0:2], pb_bc[:, 0:2]):
            nc.vector.tensor_copy(touch[:, 0:2], ap)
            nc.scalar.copy(touch[:, 2:4], ap)
        nc.vector.tensor_copy(touch[0:14, 0:2], saw_sb[:, 0:2])
        nc.scalar.copy(touch[0:14, 2:4], saw_sb[:, 0:2])

        # PE warm-up during the input-DMA wait (HAM clock gate to 8/8)
        warm = big.tile([128, 8], F32, tag="warm")
        nc.vector.memset(warm[:], 1.0)
        with tc.tile_pool(name="wrm", bufs=2, space="PSUM") as wrm:
            for i in range(90):
                wps = wrm.tile([8, 8], F32, tag="w")
                nc.tensor.matmul(wps[:], warm[:, 0:8], warm[:, 0:8],
                                 start=True, stop=True)

        # ---------- projection targets ----------
        qts = [big.tile([128, W], BF16, tag="qt%d" % t, name="qt%d" % t)
               for t in range(6)]
        kts = [big.tile([128, N], BF16, tag="kt%d" % t, name="kt%d" % t)
               for t in range(6)]
        v_sb = big.tile([128, 18, 65 * HEADS], BF16, tag="v")
        nc.vector.memset(
            v_sb[:].rearrange("p t (h x) -> p t h x", x=65)[:, :, :, 64:65], 1.0)

        def emit_k(pool, t):
            # kts[t] spans all 2304 keys across the 3 xkv chunks
            for c0 in range(0, N, DIM):
                xp = xkv_ps[c0 // DIM]
                for o0, ow in ((0, 512), (512, 256)):
                    ps = pool.tile([128, 512], F32, tag="pj")
                    for ct in range(6):
                        nc.tensor.matmul(ps[:, 0:ow],
                                         wk_sb[:, ct, 128 * t:128 * (t + 1)],
                                         xp[:, ct, o0:o0 + ow],
                                         start=(ct == 0), stop=(ct == 5))
                    nc.vector.tensor_scalar_add(
                        kts[t][:, c0 + o0:c0 + o0 + ow], ps[:, 0:ow],
                        kb_sb[:, t:t + 1])

        def emit_q(pool, t):
            for o0, ow in ((0, 512), (512, W - 512)):
                ps = pool.tile([128, 512], F32, tag="pj")
                for ct in range(6):
                    nc.tensor.matmul(ps[:, 0:ow],
                                     wq_sb[:, ct, 128 * t:128 * (t + 1)],
                                     xq_sb[:, ct, o0:o0 + ow],
                                     start=(ct == 0), stop=(ct == 5))
                nc.vector.tensor_scalar_add(qts[t][:, o0:o0 + ow],
                                            ps[:, 0:ow], qb_sb[:, t:t + 1])

        def emit_v(pool, mt):
            xp = xkv_ps[mt // 6]
            mo = 128 * (mt % 6)
            for o0, ow in ((0, 512), (512, 256)):
                ps = pool.tile([128, 512], F32, tag="pj")
                for ct in range(6):
                    nc.tensor.matmul(ps[:, 0:ow],
                                     xp[:, ct, mo:mo + 128],
                                     wv_sb[:, ct, o0:o0 + ow],
                                     start=(ct == 0), stop=(ct == 5))
                h0, h1 = o0 // 64, (o0 + ow) // 64
                nc.vector.tensor_add(
                    v_sb[:, mt].rearrange("p (h x) -> p h x",
                                          x=65)[:, h0:h1, 0:64],
                    ps[:, 0:ow].rearrange("p (h x) -> p h x", x=64),
                    vb_bc[:, o0:o0 + ow].rearrange("p (h x) -> p h x", x=64))

        # ---------- attention with projections interleaved ----------
        attn_ts = [big.tile([128, W], BF16, tag="attn%d" % t,
                            name="attn%d" % t) for t in range(6)]
        with (
            tc.tile_pool(name="pj", bufs=2, space="PSUM") as pjp,
            tc.tile_pool(name="st", bufs=2, space="PSUM") as stp,
            tc.tile_pool(name="av", bufs=1, space="PSUM") as avp,
            tc.tile_pool(name="pt", bufs=4) as ptp,
            tc.tile_pool(name="fin", bufs=1) as finp,
            tc.tile_pool(name="drd", bufs=2, space="DRAM") as drdp,
        ):
            emit_k(pjp, 0)
            emit_q(pjp, 0)
            for mt in range(18):
                emit_v(pjp, mt)
            for h in range(HEADS):
                t, bp = h // 2, 64 * (h % 2)
                if h % 2 == 0 and t > 0:
                    emit_k(pjp, t)
                    emit_q(pjp, t)
                qt_h = qts[t][bp:bp + HD, :]
                kt_h = kts[t][bp:bp + HD, :]
                av = avp.tile([65, W], F32, tag="av")
                for jt in range(18):
                    st = stp.tile([128, W], F32, tag="st")
                    lhs = kt_h[:, 128 * jt:128 * (jt + 1)]
                    nc.tensor.matmul(st[:, 0:512], lhs, qt_h[:, 0:512],
                                     start=True, stop=True)
                    nc.tensor.matmul(st[:, 512:W], lhs, qt_h[:, 512:W],
                                     start=True, stop=True)
                    pt = ptp.tile([128, W], BF16, tag="pt")
                    nc.scalar.activation(pt[:], st[:], AF.Exp, scale=SCALE)
                    vh = v_sb[:, jt, 65 * h:65 * h + 65]
                    nc.tensor.matmul(av[:, 0:512], vh, pt[:, 0:512],
                                     start=(jt == 0), stop=(jt == 17))
                    nc.tensor.matmul(av[:, 512:W], vh, pt[:, 512:W],
                                     start=(jt == 0), stop=(jt == 17))
                recip = finp.tile([1, W], F32, tag="recip")
                nc.vector.reciprocal(recip[:], av[64:65, :])
                rd = drdp.tile([1, W], F32, tag="rd")
                nc.sync.dma_start(rd[:], recip[:])
                bc = finp.tile([64, W], F32, tag="bc")
                rap = rd[:]
                nc.sync.dma_start(bc[:], bass.AP(
                    tensor=rap.tensor, offset=rap.offset,
                    ap=[[0, HD], [1, W]]))
                nc.vector.tensor_mul(attn_ts[t][bp:bp + HD, :],
                                     av[0:HD, :], bc[:])

        ins_pool.__exit__(None, None, None)

        # ---------- proj + stats ----------
        out_sb = big.tile([128, 5, DIM], F32, tag="out")
        stats = big.tile([128, 10], F32, tag="stats")
        nc.vector.memset(stats[:], 0.0)
        dcc_pool = tc.tile_pool(name="dcc", bufs=1, space="DRAM")
        dcc = dcc_pool.__enter__()
        cin = dcc.tile([2, STATC], F32, tag="cin")
        cout = dcc.tile([8, STATC], F32, tag="cout")
        with tc.tile_pool(name="pp", bufs=2, space="PSUM") as ppp:
            for it in range(5):
                iw = 128 if it < 4 else 64
                pp = ppp.tile([128, DIM], F32, tag="pp")
                for o0, ow in ((0, 512), (512, 256)):
                    for ct in range(6):
                        nc.tensor.matmul(
                            pp[:iw, o0:o0 + ow],
                            attn_ts[ct][:, 128 * it:128 * it + iw],
                            wp_sb[:, ct, o0:o0 + ow],
                            start=(ct == 0), stop=(ct == 5))
                nc.vector.tensor_add(out_sb[:iw, it, :], pp[:iw, 0:DIM],
                                     pb_bc[:iw, :])
                nc.vector.reduce_sum(stats[:iw, it:it + 1],
                                     out_sb[:iw, it, :], axis=AX.X)
                nc.vector.reduce_max(stats[:iw, 5 + it:6 + it],
                                     out_sb[:iw, it, :], axis=AX.X)

                # stats column -> row-major slot in the collective input
                # (always 128 rows: rows 64-127 of col 4/9 are zeros, which
                # also zero-fills the 576:640 pad the AllGather reads)
                for ch, col in ((0, it), (1, 5 + it)):
                    nc.sync.dma_start(
                        cin[ch, 128 * it:128 * (it + 1)]
                        .rearrange("(a b) -> a b", b=1),
                        stats[:, col:col + 1])
            nc.gpsimd.collective_compute(
                "AllGather", mybir.AluOpType.bypass,
                replica_groups=[[0, 1, 2, 3], [4, 5, 6, 7]],
                ins=[cin[:]], outs=[cout[:]])

            # gathered stats -> gutter-padded conv rows (own rows static
            # position, halo rows via partition-id-dependent offsets)
            mprime = big.tile([2, MPW], F32, tag="mp")
            nc.vector.memset(mprime[:], 0.0)
            pid = nc.sync.partition_id()
            r = pid % 4
            ct_ = cout[:].tensor
            mrow = list(mprime[0:2, 0:1].ap[0])     # partition stride, n=2
            sl = mprime[0:2, 3 * MC + 3:3 * MC + 4]
            nc.sync.dma_start(
                bass.AP(tensor=sl.tensor, offset=sl.offset,
                        ap=[mrow, [MC, ROWS_W], [1, WID]]),
                bass.AP(tensor=ct_, offset=r * 2 * STATC,
                        ap=[[STATC, 2], [1, W]]))
            slt = mprime[0:2, 3:4]
            nc.sync.dma_start(
                bass.AP(tensor=slt.tensor, offset=slt.offset,
                        ap=[mrow, [MC, 3], [1, WID]]),
                bass.AP(tensor=ct_, offset=r * 2 * STATC + 432 - 2 * STATC,
                        ap=[[STATC, 2], [1, 144]]),
                cond=(r >= 1))
            slb = mprime[0:2, 15 * MC + 3:15 * MC + 4]
            nc.sync.dma_start(
                bass.AP(tensor=slb.tensor, offset=slb.offset,
                        ap=[mrow, [MC, 3], [1, WID]]),
                bass.AP(tensor=ct_, offset=r * 2 * STATC + 2 * STATC,
                        ap=[[STATC, 2], [1, 144]]),
                cond=(r <= 2))
            # A'[(ci,ky), q] = mprime[ci, ky*MC + q]  (overlapping rows)
            aprime = big.tile([14, CONV_SPAN + 6], F32, tag="ap")
            for ci in range(2):
                for ky in range(SA_K):
                    nc.sync.dma_start(
                        aprime[ci * SA_K + ky:ci * SA_K + ky + 1, :],
                        mprime[ci:ci + 1, ky * MC:ky * MC + CONV_SPAN + 6])
            # conv = 7 shifted K=14 matmuls
            cps = ppp.tile([1, CONV_SPAN], F32, tag="cps")
            for s0, sw in ((0, 512), (512, CONV_SPAN - 512)):
                for kx in range(SA_K):
                    nc.tensor.matmul(cps[:, s0:s0 + sw],
                                     saw_sb[:, kx:kx + 1],
                                     aprime[:, kx + s0:kx + s0 + sw],
                                     start=(kx == 0), stop=(kx == 6))
            sig_row = big.tile([1, CONV_SPAN], F32, tag="sigr")
            nc.scalar.activation(sig_row[:], cps[:], AF.Sigmoid)
            sig_clean = big.tile([1, W], F32, tag="sigc")
            sr = sig_row[:, 0:WID]
            sig_src = bass.AP(tensor=sr.tensor, offset=sr.offset,
                              ap=[list(sr.ap[0]), [MC, ROWS_W], [1, WID]])
            nc.vector.tensor_copy(
                sig_clean[:].rearrange("p (r c) -> p r c", c=WID), sig_src)
            sc_s = dcc.tile([1, W], F32, tag="scs")
            nc.sync.dma_start(sc_s[:], sig_clean[:])
            sig_col = big.tile([128, 5], F32, tag="sigcol")
            nc.vector.memset(sig_col[:], 0.0)
            nc.sync.dma_start(
                sig_col[:, 0:4],
                sc_s[0, 0:512].rearrange("(b a) -> a b", b=4))
            nc.sync.dma_start(
                sig_col[0:64, 4:5],
                sc_s[0, 512:W].rearrange("(a b) -> a b", b=1))
            dcc_pool.__exit__(None, None, None)
            for it in range(5):
                iw = 128 if it < 4 else 64
                nc.vector.tensor_scalar_mul(out_sb[:iw, it, :],
                                            out_sb[:iw, it, :],
                                            sig_col[:iw, it:it + 1])
                if it < 4:
                    nc.sync.dma_start(
                        out_d[128 * it:128 * (it + 1)], out_sb[:, it, :])
                else:
                    nc.sync.dma_start(out_d[512:W], out_sb[0:64, 4, :])


_NC = None
LAST_RESULTS = None


def _get_nc():
    global _NC
    if _NC is None:
        _NC = build_program()
    return _NC


def make_in_maps(q_input, kv_input, q_w, q_b, kv_w, kv_b, proj_w, proj_b,
                 sa_w):
    f32 = np.float32
    q_input = np.asarray(q_input, f32)
    kv_input = np.asarray(kv_input, f32)
    wq_t = np.ascontiguousarray(np.asarray(q_w, f32).T).astype(bf16)
    wk_t = np.ascontiguousarray(np.asarray(kv_w, f32)[:DIM].T).astype(bf16)
    wv_t = np.ascontiguousarray(np.asarray(kv_w, f32)[DIM:].T).astype(bf16)
    wp_t = np.ascontiguousarray(np.asarray(proj_w, f32).T).astype(bf16)
    qb = np.asarray(q_b, f32).reshape(6, 128)
    kb = np.asarray(kv_b, f32)[:DIM].reshape(6, 128)
    vb = np.asarray(kv_b, f32)[DIM:].reshape(1, DIM)
    pb = np.asarray(proj_b, f32).reshape(1, DIM)
    sa = np.asarray(sa_w, f32)[0].copy()          # [2, 7, 7]
    sa[0] /= DIM                                  # fold 1/768 mean scale
    saw = np.ascontiguousarray(sa.reshape(14, SA_K))

    shared = dict(wq_t=wq_t, wk_t=wk_t, wv_t=wv_t, wp_t=wp_t,
                  qb=qb, kb=kb, vb=vb, pb=pb, saw=saw)
    in_maps = []
    for b in range(B):
        xkv = np.ascontiguousarray(kv_input[b].T).astype(bf16)
        for c in range(4):
            xq = np.ascontiguousarray(
                q_input[b, W * c:W * (c + 1)].T).astype(bf16)
            in_maps.append(dict(xq_t=xq, xkv_t=xkv, **shared))
    return in_maps


def kernel(q_input, kv_input, q_w, q_b, kv_w, kv_b, proj_w, proj_b, sa_w):
    f32 = np.float32
    in_maps = make_in_maps(q_input, kv_input, q_w, q_b, kv_w, kv_b,
                           proj_w, proj_b, sa_w)
    res = run_bass_kernel_spmd(_get_nc(), in_maps, core_ids=list(range(8)))
    global LAST_RESULTS
    LAST_RESULTS = res
    out = np.zeros((B, N, DIM), dtype=f32)
    for b in range(B):
        for c in range(4):
            out[b, W * c:W * (c + 1)] = res.results[4 * b + c]["out"]
    return out

